# revision 5
# baseline (speedup 1.0000x reference)
"""Chamfer L1 distance kernel for Trainium2 (8 NeuronCores) — sorted-window
algorithm.

Full inputs: pred [4, 8192, 3] f32, target [4, 8192, 3] f32.
Output: scalar f32 = mean over batch of (sum_i min_j d(i,j) + sum_j min_i d(i,j)),
d = L1 distance.

Algorithm (exact, with on-host verification + fallback):
  d(p,t) = |dx|+|dy|+|dz| >= |u_p - u_t| with u = x+y+z.  Sort preds and
  targets of each batch by u.  A pred at sorted rank g only needs to scan
  targets in a rank window centered at g (counts match, so ranks align by
  quantile); any target outside the window is at u-distance >= the window
  edge gap.  After the device pass, the host checks every returned min m
  against its window-edge u-gap; the ~0.4% of points whose NN might lie
  outside their window (locally sparse regions) are recomputed exactly on
  host.  Device mins are upper bounds, so the check is sound.

Sharding: 8 cores = 4 batches x 2 pred-halves (sorted rank split).  Each core:
32 blocks of 128 preds x K-wide target window (window slides 128 ranks per
block).  Device outputs per core:
  - rowmin [128, 32] bf16: min over the pred's window
  - colmin [128, W] bf16 (W = 4096+K-128): running min over this core's pred
    blocks for each covered target rank (partition p = pred lane; host
    reduces over partitions/cores)
Engine split per block b (f32 T tiles resident in SBUF, bf16 intermediates):
  ACT: A_d = |T_d - p_d|  (Abs activation, bias=-p_d; subtract in f32 then
       round to bf16 — no cancellation); on odd blocks |Dz| moves to DVE
       (raw diff + negate + max) to balance engine load.
  DVE: S01 = A0+A1, S = S01+A2 (TT add 2x), colmin = min(colmin, S),
       rowmin: fold K->K/2->K/4 (TT min) then tensor_reduce.
"""

import sys

sys.path.insert(0, "/opt/trn_rl_repo")

import numpy as np

N_CORES = 8
B, N, M = 4, 8192, 8192
P = 128
NPRED = N // 2  # preds per core
NBLK = NPRED // P  # 32
K = 512  # target window width (ranks)
W = NPRED + K - P  # colmin/target tile width per core
SENTINEL = 30000.0
BIG = 60000.0
ALT_Z = 1  # every ALT_Z-th block computes |Dz| on DVE instead of ACT (0 = never)
ALT_Y = 0  # every ALT_Y-th block computes |Dy| on DVE instead of ACT (0 = never)

_compiled = None


def _build(reps=1):
    import concourse.bacc as bacc
    import concourse.mybir as mybir
    import concourse.tile as tile

    f32 = mybir.dt.float32
    bf16 = mybir.dt.bfloat16
    Alu = mybir.AluOpType
    Act = mybir.ActivationFunctionType

    nc = bacc.Bacc("TRN2", debug=False, num_devices=N_CORES)
    pred_rn = nc.dram_tensor("pred_rn", [P, NBLK * 3], f32, kind="ExternalInput").ap()
    target_t = nc.dram_tensor("target_t", [3, W], f32, kind="ExternalInput").ap()
    rowmin_d = nc.dram_tensor("rowmin", [P, NBLK], bf16, kind="ExternalOutput").ap()
    colmin_d = nc.dram_tensor("colmin", [P, W], bf16, kind="ExternalOutput").ap()

    with tile.TileContext(nc) as tc:
        with (
            tc.tile_pool(name="const", bufs=1) as cpool,
            tc.tile_pool(name="apool", bufs=4) as apool,
            tc.tile_pool(name="wpool", bufs=4) as wpool,
        ):
            PNt = cpool.tile([P, NBLK * 3], f32, tag="PN")
            nc.sync.dma_start(PNt[:, :], pred_rn[:, :])

            T = [cpool.tile([P, W], f32, tag=f"T{d}", name=f"T{d}") for d in range(3)]
            for d in range(3):
                nc.sync.dma_start(
                    T[d][:, :], target_t[d : d + 1, :].broadcast_to([P, W])
                )

            colmin = cpool.tile([P, W], bf16, tag="colmin")
            nc.vector.memset(colmin[:, :], BIG)
            rowmin = cpool.tile([P, NBLK], bf16, tag="rowmin")

            import contextlib

            loop_ctx = tc.For_i(0, reps, 1) if reps > 1 else contextlib.nullcontext()
            with loop_ctx:
                for r in range(NBLK):
                    ws = slice(P * r, P * r + K)
                    biases = [PNt[:, 3 * r + d : 3 * r + d + 1] for d in range(3)]
                    z_on_dve = ALT_Z > 0 and (r % ALT_Z == ALT_Z - 1)
                    y_on_dve = ALT_Y > 0 and (r % ALT_Y == ALT_Y - 1)

                    def make_abs(d, on_dve, tag):
                        Ad = apool.tile([P, K], bf16, tag=tag, name=tag)
                        if on_dve:
                            Dd = wpool.tile([P, K], bf16, tag=f"D{tag}", name=f"D{tag}")
                            nc.vector.tensor_scalar_add(
                                Dd[:, :], T[d][:, ws], biases[d]
                            )
                            nDd = wpool.tile(
                                [P, K], bf16, tag=f"nD{tag}", name=f"nD{tag}"
                            )
                            nc.vector.tensor_scalar_mul(nDd[:, :], Dd[:, :], -1.0)
                            nc.vector.tensor_tensor(
                                Ad[:, :], Dd[:, :], nDd[:, :], Alu.max
                            )
                        else:
                            nc.scalar.activation(
                                Ad[:, :], T[d][:, ws], Act.Abs, bias=biases[d], scale=1.0
                            )
                        return Ad

                    A0 = make_abs(0, False, "A0")
                    A1 = make_abs(1, y_on_dve, "A1")
                    A2 = make_abs(2, z_on_dve, "A2")
                    S01 = wpool.tile([P, K], bf16, tag="S01")
                    nc.vector.tensor_tensor(S01[:, :], A0[:, :], A1[:, :], Alu.add)
                    S = wpool.tile([P, K], bf16, tag="S")
                    nc.vector.tensor_tensor(S[:, :], S01[:, :], A2[:, :], Alu.add)
                    nc.vector.tensor_tensor(
                        colmin[:, ws], colmin[:, ws], S[:, :], Alu.min
                    )
                    F1 = wpool.tile([P, K // 2], bf16, tag="F1")
                    nc.vector.tensor_tensor(
                        F1[:, :], S[:, : K // 2], S[:, K // 2 :], Alu.min
                    )
                    F2 = wpool.tile([P, K // 4], bf16, tag="F2")
                    nc.vector.tensor_tensor(
                        F2[:, :], F1[:, : K // 4], F1[:, K // 4 :], Alu.min
                    )
                    nc.vector.tensor_reduce(
                        rowmin[:, r : r + 1], F2[:, :], mybir.AxisListType.X, Alu.min
                    )

            nc.sync.dma_start(rowmin_d[:, :], rowmin[:, :])
            nc.sync.dma_start(colmin_d[:, :], colmin[:, :])

    nc.compile()
    return nc


def _sort_batch(pred_b, target_b):
    up = pred_b.sum(1)
    ut = target_b.sum(1)
    po = np.argsort(up, kind="stable")
    to = np.argsort(ut, kind="stable")
    return pred_b[po], target_b[to], up[po], ut[to]


def _shard(pred, target):
    HALF = K // 2
    in_maps = []
    meta = []
    for b in range(B):
        ps, ts, ups, uts = _sort_batch(pred[b], target[b])
        meta.append((ps, ts, ups, uts))
        for h in range(2):
            pr = ps[h * NPRED : (h + 1) * NPRED]  # [4096, 3]
            prn = np.ascontiguousarray(
                -pr.reshape(NBLK, P, 3).transpose(1, 0, 2).reshape(P, NBLK * 3)
            )
            G0 = NPRED * h + P // 2 - HALF
            Tpad = np.full((W, 3), SENTINEL, np.float32)
            lo, hi = max(0, G0), min(M, G0 + W)
            Tpad[lo - G0 : hi - G0] = ts[lo:hi]
            tt = np.ascontiguousarray(Tpad.T)  # [3, W]
            in_maps.append({"pred_rn": prn, "target_t": tt})
    return in_maps, meta


def _combine(results, meta):
    HALF = K // 2
    total = 0.0
    for b in range(B):
        ps, ts, ups, uts = meta[b]
        m_row = np.full(N, np.inf, np.float32)
        m_col = np.full(M, np.inf, np.float32)
        covA = np.full(M, N, np.int64)
        covB = np.full(M, -1, np.int64)
        for h in range(2):
            r = results[2 * b + h]
            rm = np.asarray(r["rowmin"]).astype(np.float32)  # [128, 32]
            # rowmin[p, blk] = pred at sorted rank NPRED*h + 128*blk + p
            gidx = NPRED * h + P * np.arange(NBLK)[None, :] + np.arange(P)[:, None]
            m_row[gidx.ravel()] = rm.ravel()
            cm = np.asarray(r["colmin"]).astype(np.float32).min(axis=0)  # [W]
            G0 = NPRED * h + P // 2 - HALF
            gt = G0 + np.arange(W)
            valid = (gt >= 0) & (gt < M)
            np.minimum.at(m_col, gt[valid], cm[valid])
            # coverage: block blk covers targets [G0+128*blk, G0+128*blk+K)
            for blk in range(NBLK):
                g = G0 + P * blk + np.arange(K)
                v = (g >= 0) & (g < M)
                covA[g[v]] = np.minimum(covA[g[v]], NPRED * h + P * blk)
                covB[g[v]] = np.maximum(covB[g[v]], NPRED * h + P * blk + P)
        # verification: rowmin
        h_arr = np.arange(N) // NPRED
        r_arr = (np.arange(N) % NPRED) // P
        wlo = NPRED * h_arr + P * r_arr + P // 2 - HALF
        whi = wlo + K
        gap_lo = np.where(wlo > 0, ups - uts[np.clip(wlo, 1, M) - 1], np.inf)
        gap_hi = np.where(whi < M, uts[np.clip(whi, 0, M - 1)] - ups, np.inf)
        ok_r = m_row <= np.minimum(gap_lo, gap_hi)
        for g in np.where(~ok_r)[0]:
            m_row[g] = np.abs(ps[g][None, :] - ts).sum(1).min()
        # verification: colmin
        gap_lo_c = np.where(covA > 0, uts - ups[np.clip(covA, 1, N) - 1], np.inf)
        gap_hi_c = np.where(covB < N, ups[np.clip(covB, 0, N - 1)] - uts, np.inf)
        ok_c = (m_col <= np.minimum(gap_lo_c, gap_hi_c)) & (covB > covA)
        for j in np.where(~ok_c)[0]:
            m_col[j] = np.abs(ts[j][None, :] - ps).sum(1).min()
        total += m_row.sum(dtype=np.float64) + m_col.sum(dtype=np.float64)
    return np.float32(total / B)


def kernel(pred, target):
    global _compiled
    from concourse import bass_utils

    pred = np.asarray(pred, dtype=np.float32)
    target = np.asarray(target, dtype=np.float32)
    if _compiled is None:
        _compiled = _build()
    in_maps, meta = _shard(pred, target)
    res = bass_utils.run_bass_kernel_spmd(
        _compiled, in_maps, core_ids=list(range(N_CORES))
    )
    return _combine(res.results, meta)


# revision 11
# speedup vs baseline: 1.4976x; 1.4976x over previous
"""Chamfer L1 distance kernel for Trainium2 (8 NeuronCores) — sorted-window
algorithm.

Full inputs: pred [4, 8192, 3] f32, target [4, 8192, 3] f32.
Output: scalar f32 = mean over batch of (sum_i min_j d(i,j) + sum_j min_i d(i,j)),
d = L1 distance.

Algorithm (exact, with on-host verification + fallback):
  d(p,t) = |dx|+|dy|+|dz| >= |u_p - u_t| with u = x+y+z.  Sort preds and
  targets of each batch by u.  A pred at sorted rank g only needs to scan
  targets in a rank window centered at g (counts match, so ranks align by
  quantile); any target outside the window is at u-distance >= the window
  edge gap.  After the device pass, the host checks every returned min m
  against its window-edge u-gap; the ~0.4% of points whose NN might lie
  outside their window (locally sparse regions) are recomputed exactly on
  host.  Device mins are upper bounds, so the check is sound.

Sharding: 8 cores = 4 batches x 2 pred-halves (sorted rank split).  Each core:
32 blocks of 128 preds x K-wide target window (window slides 128 ranks per
block).  Device outputs per core:
  - rowmin [128, 32] bf16: min over the pred's window
  - colmin [128, W] bf16 (W = 4096+K-128): running min over this core's pred
    blocks for each covered target rank (partition p = pred lane; host
    reduces over partitions/cores)
Engine split per block b (f32 T tiles resident in SBUF, bf16 intermediates):
  ACT: A_d = |T_d - p_d|  (Abs activation, bias=-p_d; subtract in f32 then
       round to bf16 — no cancellation); on odd blocks |Dz| moves to DVE
       (raw diff + negate + max) to balance engine load.
  DVE: S01 = A0+A1, S = S01+A2 (TT add 2x), colmin = min(colmin, S),
       rowmin: fold K->K/2->K/4 (TT min) then tensor_reduce.
"""

import sys

sys.path.insert(0, "/opt/trn_rl_repo")

import numpy as np

N_CORES = 8
B, N, M = 4, 8192, 8192
P = 128
NPRED = N // 2  # preds per core
NBLK = NPRED // P  # 32
K = 512  # target window width (ranks)
W = NPRED + K - P  # colmin/target tile width per core
SENTINEL = 30000.0
BIG = 60000.0
ALT_Z = 1  # every ALT_Z-th block computes |Dz| on DVE instead of ACT (0 = never)
ALT_Y = 0  # every ALT_Y-th block computes |Dy| on DVE instead of ACT (0 = never)
ROWMIN_MODE = "fold2"  # fold2 | fold1 | direct | scan
OPSET = "fused"  # classic | fused

_compiled = None
_chamfer_ops = None


def _register_ops():
    """Register the two fused chamfer DVE ops (runtime extension of the
    custom-DVE registry; uop tables are emitted per-NEFF at compile time).

    CHAMFER_ABS2_SUM:    out = |in0 + s0| + |in1 + s1|          (s = -pred coord)
    CHAMFER_ABS1_ADD_MIN: out = |in0 + s0| + in1;  accum_out = min(out) seeded s1
    """
    global _chamfer_ops
    if _chamfer_ops is not None:
        return _chamfer_ops
    import numpy as np
    import concourse.dve_ops as dve_ops
    from concourse.dve_ops import DveOp
    from concourse.dve_spec import Spec, Src0, Src1, C0, C1, Zero, maxx, minn, lower
    from concourse.dve_spec import _has_src1
    from concourse.dve_uop import DveOpSpec

    d0 = Src0 + C0
    d1 = Src1 + C1
    spec1 = Spec(
        body=maxx(d0, Zero - d0) + maxx(d1, Zero - d1),
        reference=lambda in0, in1, s0, s1, imm2: (
            np.abs(in0.astype(np.float32) + s0) + np.abs(in1 + s1)
        ),
    )

    def _ref2(in0, in1, s0, s1, imm2):
        out = (np.abs(in0.astype(np.float32) + s0) + in1).astype(np.float32)
        acc = np.minimum(out.reshape(out.shape[0], -1).min(-1, keepdims=True), s1)
        return out, acc

    dz = Src0 + C0
    spec2 = Spec(
        body=maxx(dz, Zero - dz) + Src1, accum=minn, accum_init=C1, reference=_ref2
    )

    ops = []
    for name, spec in (
        ("CHAMFER_ABS2_SUM", spec1),
        ("CHAMFER_ABS1_ADD_MIN", spec2),
    ):
        if name in dve_ops._SUB_OPCODE_FOR_NAME:
            ops.append(next(o for o in dve_ops.OPS if o.name == name))
            continue
        row = max(dve_ops._SUB_OPCODE_FOR_NAME.values()) + 1
        assert row < 0x20
        shas = {}
        for ver in ("v3", "v4"):
            try:
                shas[ver] = DveOpSpec(
                    name=name, opcode=row, uops=lower(spec, ver=ver),
                    rd1_en=_has_src1(spec),
                ).sha(ver)
            except Exception:
                pass
        op = DveOp(name, spec, subdim=False, uops_sha=shas)
        dve_ops.OPS.append(op)
        dve_ops.CUSTOM_DVE_SPECS[name] = spec
        dve_ops._SUB_OPCODE_FOR_NAME[name] = row
        ops.append(op)
    _chamfer_ops = tuple(ops)
    return _chamfer_ops


def _build(reps=1, nblocks=None):
    import concourse.bacc as bacc
    import concourse.mybir as mybir
    import concourse.tile as tile

    f32 = mybir.dt.float32
    bf16 = mybir.dt.bfloat16
    Alu = mybir.AluOpType
    Act = mybir.ActivationFunctionType

    nc = bacc.Bacc("TRN2", debug=False, num_devices=N_CORES)
    pred_rn = nc.dram_tensor("pred_rn", [P, NBLK * 3], f32, kind="ExternalInput").ap()
    target_t = nc.dram_tensor("target_t", [3, W], f32, kind="ExternalInput").ap()
    rowmin_d = nc.dram_tensor("rowmin", [P, NBLK], bf16, kind="ExternalOutput").ap()
    colmin_d = nc.dram_tensor("colmin", [P, W], bf16, kind="ExternalOutput").ap()

    with tile.TileContext(nc) as tc:
        with (
            tc.tile_pool(name="const", bufs=1) as cpool,
            tc.tile_pool(name="apool", bufs=4) as apool,
            tc.tile_pool(name="wpool", bufs=4) as wpool,
        ):
            PNt = cpool.tile([P, NBLK * 3], f32, tag="PN")
            nc.sync.dma_start(PNt[:, :], pred_rn[:, :])

            T = [cpool.tile([P, W], f32, tag=f"T{d}", name=f"T{d}") for d in range(3)]
            for d in range(3):
                nc.sync.dma_start(
                    T[d][:, :], target_t[d : d + 1, :].broadcast_to([P, W])
                )

            colmin = cpool.tile([P, W], bf16, tag="colmin")
            nc.vector.memset(colmin[:, :], BIG)
            rowmin = cpool.tile([P, NBLK], bf16, tag="rowmin")

            import contextlib

            if OPSET == "fused":
                OP1, OP2 = _register_ops()

            loop_ctx = tc.For_i(0, reps, 1) if reps > 1 else contextlib.nullcontext()
            with loop_ctx:
                for r in range(NBLK if nblocks is None else nblocks):
                    ws = slice(P * r, P * r + K)
                    biases = [PNt[:, 3 * r + d : 3 * r + d + 1] for d in range(3)]
                    if OPSET == "fused":
                        A01 = wpool.tile([P, K], bf16, tag="A01")
                        nc.vector._custom_dve(
                            OP1, out=A01[:, :], in0=T[0][:, ws], in1=T[1][:, ws],
                            s0=biases[0], s1=biases[1],
                        )
                        S = wpool.tile([P, K], bf16, tag="S")
                        nc.vector._custom_dve(
                            OP2, out=S[:, :], in0=T[2][:, ws], in1=A01[:, :],
                            s0=biases[2], s1=BIG,
                            accum_out=rowmin[:, r : r + 1],
                        )
                        nc.vector.tensor_tensor(
                            colmin[:, ws], colmin[:, ws], S[:, :], Alu.min
                        )
                        continue
                    z_on_dve = ALT_Z > 0 and (r % ALT_Z == ALT_Z - 1)
                    y_on_dve = ALT_Y > 0 and (r % ALT_Y == ALT_Y - 1)

                    def make_abs(d, on_dve, tag):
                        Ad = apool.tile([P, K], bf16, tag=tag, name=tag)
                        if on_dve:
                            Dd = wpool.tile([P, K], bf16, tag=f"D{tag}", name=f"D{tag}")
                            nc.vector.tensor_scalar_add(
                                Dd[:, :], T[d][:, ws], biases[d]
                            )
                            nDd = wpool.tile(
                                [P, K], bf16, tag=f"nD{tag}", name=f"nD{tag}"
                            )
                            nc.vector.tensor_scalar_mul(nDd[:, :], Dd[:, :], -1.0)
                            nc.vector.tensor_tensor(
                                Ad[:, :], Dd[:, :], nDd[:, :], Alu.max
                            )
                        else:
                            nc.scalar.activation(
                                Ad[:, :], T[d][:, ws], Act.Abs, bias=biases[d], scale=1.0
                            )
                        return Ad

                    A0 = make_abs(0, False, "A0")
                    A1 = make_abs(1, y_on_dve, "A1")
                    A2 = make_abs(2, z_on_dve, "A2")
                    S01 = wpool.tile([P, K], bf16, tag="S01")
                    nc.vector.tensor_tensor(S01[:, :], A0[:, :], A1[:, :], Alu.add)
                    S = wpool.tile([P, K], bf16, tag="S")
                    nc.vector.tensor_tensor(S[:, :], S01[:, :], A2[:, :], Alu.add)
                    nc.vector.tensor_tensor(
                        colmin[:, ws], colmin[:, ws], S[:, :], Alu.min
                    )
                    if ROWMIN_MODE == "direct":
                        nc.vector.tensor_reduce(
                            rowmin[:, r : r + 1], S[:, :], mybir.AxisListType.X, Alu.min
                        )
                    elif ROWMIN_MODE == "scan":
                        Osc = wpool.tile([P, K // 2], bf16, tag="Osc")
                        nc.vector.tensor_tensor_scan(
                            Osc[:, :], S[:, : K // 2], S[:, K // 2 :], BIG,
                            Alu.min, Alu.min,
                        )
                        nc.vector.tensor_copy(
                            rowmin[:, r : r + 1], Osc[:, K // 2 - 1 : K // 2]
                        )
                    else:
                        F1 = wpool.tile([P, K // 2], bf16, tag="F1")
                        nc.vector.tensor_tensor(
                            F1[:, :], S[:, : K // 2], S[:, K // 2 :], Alu.min
                        )
                        if ROWMIN_MODE == "fold1":
                            nc.vector.tensor_reduce(
                                rowmin[:, r : r + 1], F1[:, :],
                                mybir.AxisListType.X, Alu.min,
                            )
                        else:
                            F2 = wpool.tile([P, K // 4], bf16, tag="F2")
                            nc.vector.tensor_tensor(
                                F2[:, :], F1[:, : K // 4], F1[:, K // 4 :], Alu.min
                            )
                            nc.vector.tensor_reduce(
                                rowmin[:, r : r + 1], F2[:, :],
                                mybir.AxisListType.X, Alu.min,
                            )

            nc.sync.dma_start(rowmin_d[:, :], rowmin[:, :])
            nc.sync.dma_start(colmin_d[:, :], colmin[:, :])

    nc.compile()
    return nc


def _sort_batch(pred_b, target_b):
    up = pred_b.sum(1)
    ut = target_b.sum(1)
    po = np.argsort(up, kind="stable")
    to = np.argsort(ut, kind="stable")
    return pred_b[po], target_b[to], up[po], ut[to]


def _shard(pred, target):
    HALF = K // 2
    in_maps = []
    meta = []
    for b in range(B):
        ps, ts, ups, uts = _sort_batch(pred[b], target[b])
        meta.append((ps, ts, ups, uts))
        for h in range(2):
            pr = ps[h * NPRED : (h + 1) * NPRED]  # [4096, 3]
            prn = np.ascontiguousarray(
                -pr.reshape(NBLK, P, 3).transpose(1, 0, 2).reshape(P, NBLK * 3)
            )
            G0 = NPRED * h + P // 2 - HALF
            Tpad = np.full((W, 3), SENTINEL, np.float32)
            lo, hi = max(0, G0), min(M, G0 + W)
            Tpad[lo - G0 : hi - G0] = ts[lo:hi]
            tt = np.ascontiguousarray(Tpad.T)  # [3, W]
            in_maps.append({"pred_rn": prn, "target_t": tt})
    return in_maps, meta


def _combine(results, meta):
    HALF = K // 2
    total = 0.0
    for b in range(B):
        ps, ts, ups, uts = meta[b]
        m_row = np.full(N, np.inf, np.float32)
        m_col = np.full(M, np.inf, np.float32)
        covA = np.full(M, N, np.int64)
        covB = np.full(M, -1, np.int64)
        for h in range(2):
            r = results[2 * b + h]
            rm = np.asarray(r["rowmin"]).astype(np.float32)  # [128, 32]
            # rowmin[p, blk] = pred at sorted rank NPRED*h + 128*blk + p
            gidx = NPRED * h + P * np.arange(NBLK)[None, :] + np.arange(P)[:, None]
            m_row[gidx.ravel()] = rm.ravel()
            cm = np.asarray(r["colmin"]).astype(np.float32).min(axis=0)  # [W]
            G0 = NPRED * h + P // 2 - HALF
            gt = G0 + np.arange(W)
            valid = (gt >= 0) & (gt < M)
            np.minimum.at(m_col, gt[valid], cm[valid])
            # coverage: block blk covers targets [G0+128*blk, G0+128*blk+K)
            for blk in range(NBLK):
                g = G0 + P * blk + np.arange(K)
                v = (g >= 0) & (g < M)
                covA[g[v]] = np.minimum(covA[g[v]], NPRED * h + P * blk)
                covB[g[v]] = np.maximum(covB[g[v]], NPRED * h + P * blk + P)
        # verification: rowmin
        h_arr = np.arange(N) // NPRED
        r_arr = (np.arange(N) % NPRED) // P
        wlo = NPRED * h_arr + P * r_arr + P // 2 - HALF
        whi = wlo + K
        gap_lo = np.where(wlo > 0, ups - uts[np.clip(wlo, 1, M) - 1], np.inf)
        gap_hi = np.where(whi < M, uts[np.clip(whi, 0, M - 1)] - ups, np.inf)
        ok_r = m_row <= np.minimum(gap_lo, gap_hi)
        for g in np.where(~ok_r)[0]:
            m_row[g] = np.abs(ps[g][None, :] - ts).sum(1).min()
        # verification: colmin
        gap_lo_c = np.where(covA > 0, uts - ups[np.clip(covA, 1, N) - 1], np.inf)
        gap_hi_c = np.where(covB < N, ups[np.clip(covB, 0, N - 1)] - uts, np.inf)
        ok_c = (m_col <= np.minimum(gap_lo_c, gap_hi_c)) & (covB > covA)
        for j in np.where(~ok_c)[0]:
            m_col[j] = np.abs(ts[j][None, :] - ps).sum(1).min()
        total += m_row.sum(dtype=np.float64) + m_col.sum(dtype=np.float64)
    return np.float32(total / B)


def kernel(pred, target):
    global _compiled
    from concourse import bass_utils

    pred = np.asarray(pred, dtype=np.float32)
    target = np.asarray(target, dtype=np.float32)
    if _compiled is None:
        _compiled = _build()
    in_maps, meta = _shard(pred, target)
    res = bass_utils.run_bass_kernel_spmd(
        _compiled, in_maps, core_ids=list(range(N_CORES))
    )
    return _combine(res.results, meta)


# revision 35
# speedup vs baseline: 3.5454x; 2.3674x over previous
"""Chamfer L1 distance kernel for Trainium2 (8 NeuronCores) — sorted-window
algorithm.

Full inputs: pred [4, 8192, 3] f32, target [4, 8192, 3] f32.
Output: scalar f32 = mean over batch of (sum_i min_j d(i,j) + sum_j min_i d(i,j)),
d = L1 distance.

Algorithm (exact, with on-host verification + fallback):
  d(p,t) = |dx|+|dy|+|dz| >= |u_p - u_t| with u = x+y+z.  Sort preds and
  targets of each batch by u.  A pred at sorted rank g only needs to scan
  targets in a rank window centered at g (counts match, so ranks align by
  quantile); any target outside the window is at u-distance >= the window
  edge gap.  After the device pass, the host checks every returned min m
  against its window-edge u-gap; the ~0.4% of points whose NN might lie
  outside their window (locally sparse regions) are recomputed exactly on
  host.  Device mins are upper bounds, so the check is sound.

Sharding: 8 cores = 4 batches x 2 pred-halves (sorted rank split).  Each core:
32 blocks of 128 preds x K-wide target window (window slides 128 ranks per
block).  Device outputs per core:
  - rowmin [128, 32] bf16: min over the pred's window
  - colmin [128, W] bf16 (W = 4096+K-128): running min over this core's pred
    blocks for each covered target rank (partition p = pred lane; host
    reduces over partitions/cores)
Engine split per block b (f32 T tiles resident in SBUF, bf16 intermediates):
  ACT: A_d = |T_d - p_d|  (Abs activation, bias=-p_d; subtract in f32 then
       round to bf16 — no cancellation); on odd blocks |Dz| moves to DVE
       (raw diff + negate + max) to balance engine load.
  DVE: S01 = A0+A1, S = S01+A2 (TT add 2x), colmin = min(colmin, S),
       rowmin: fold K->K/2->K/4 (TT min) then tensor_reduce.
"""

import sys

sys.path.insert(0, "/opt/trn_rl_repo")

import numpy as np

N_CORES = 8
B, N, M = 4, 8192, 8192
P = 128
NPRED = N // 2  # preds per core
NBLK = NPRED // P  # 32
K = 320  # target window width (ranks)
W = NPRED + K - P  # colmin/target tile width per core
SENTINEL = 30000.0
BIG = 60000.0
ALT_Z = 1  # every ALT_Z-th block computes |Dz| on DVE instead of ACT (0 = never)
ALT_Y = 0  # every ALT_Y-th block computes |Dy| on DVE instead of ACT (0 = never)
ROWMIN_MODE = "fold2"  # fold2 | fold1 | direct | scan
OPSET = "fused5"  # classic | fused | fused2 | fused3 | fused4 | fused5
NSHEET = 3  # fused3/4: number of rotating colmin sheets; NSHEET*P must be >= K
FMIX = 3  # fused4/5: every FMIX-th block (pair) runs DVE-only (op1 path); 0 = never

_compiled = None
_chamfer_ops = None


def _register_ops():
    """Register the two fused chamfer DVE ops (runtime extension of the
    custom-DVE registry; uop tables are emitted per-NEFF at compile time).

    CHAMFER_ABS2_SUM:    out = |in0 + s0| + |in1 + s1|          (s = -pred coord)
    CHAMFER_ABS1_ADD_MIN: out = |in0 + s0| + in1;  accum_out = min(out) seeded s1
    """
    global _chamfer_ops
    if _chamfer_ops is not None:
        return _chamfer_ops
    import numpy as np
    import concourse.dve_ops as dve_ops
    from concourse.dve_ops import DveOp
    from concourse.dve_spec import Spec, Src0, Src1, C0, C1, Zero, maxx, minn, lower
    from concourse.dve_spec import _has_src1
    from concourse.dve_uop import DveOpSpec

    d0 = Src0 + C0
    d1 = Src1 + C1
    spec1 = Spec(
        body=maxx(d0, Zero - d0) + maxx(d1, Zero - d1),
        reference=lambda in0, in1, s0, s1, imm2: (
            np.abs(in0.astype(np.float32) + s0) + np.abs(in1 + s1)
        ),
    )

    def _ref2(in0, in1, s0, s1, imm2):
        out = (np.abs(in0.astype(np.float32) + s0) + in1).astype(np.float32)
        acc = np.minimum(out.reshape(out.shape[0], -1).min(-1, keepdims=True), s1)
        return out, acc

    dz = Src0 + C0
    spec2 = Spec(
        body=maxx(dz, Zero - dz) + Src1, accum=minn, accum_init=C1, reference=_ref2
    )

    ops = []
    for name, spec in (
        ("CHAMFER_ABS2_SUM", spec1),
        ("CHAMFER_ABS1_ADD_MIN", spec2),
    ):
        if name in dve_ops._SUB_OPCODE_FOR_NAME:
            ops.append(next(o for o in dve_ops.OPS if o.name == name))
            continue
        row = max(dve_ops._SUB_OPCODE_FOR_NAME.values()) + 1
        assert row < 0x20
        shas = {}
        for ver in ("v3", "v4"):
            try:
                shas[ver] = DveOpSpec(
                    name=name, opcode=row, uops=lower(spec, ver=ver),
                    rd1_en=_has_src1(spec),
                ).sha(ver)
            except Exception:
                pass
        op = DveOp(name, spec, subdim=False, uops_sha=shas)
        dve_ops.OPS.append(op)
        dve_ops.CUSTOM_DVE_SPECS[name] = spec
        dve_ops._SUB_OPCODE_FOR_NAME[name] = row
        ops.append(op)
    _chamfer_ops = tuple(ops)
    return _chamfer_ops


def _build(reps=1, nblocks=None):
    import concourse.bacc as bacc
    import concourse.mybir as mybir
    import concourse.tile as tile

    f32 = mybir.dt.float32
    bf16 = mybir.dt.bfloat16
    Alu = mybir.AluOpType
    Act = mybir.ActivationFunctionType

    nc = bacc.Bacc("TRN2", debug=False, num_devices=N_CORES)
    pred_rn = nc.dram_tensor("pred_rn", [P, NBLK * 3], f32, kind="ExternalInput").ap()
    target_t = nc.dram_tensor("target_t", [3, W], f32, kind="ExternalInput").ap()
    rowmin_d = nc.dram_tensor("rowmin", [P, NBLK], bf16, kind="ExternalOutput").ap()
    if OPSET in ("fused3", "fused4", "fused5"):
        assert NSHEET * P >= K
        sheet_d = [
            nc.dram_tensor(f"colmin{s}", [P, W], bf16, kind="ExternalOutput").ap()
            for s in range(NSHEET)
        ]
    else:
        colmin_d = nc.dram_tensor("colmin", [P, W], bf16, kind="ExternalOutput").ap()
    if OPSET == "fused5":
        ident_d = nc.dram_tensor("ident", [P, P], bf16, kind="ExternalInput").ap()

    with tile.TileContext(nc) as tc:
        with (
            tc.tile_pool(name="const", bufs=1) as cpool,
            tc.tile_pool(name="apool", bufs=4) as apool,
            tc.tile_pool(name="wpool", bufs=8) as wpool,
            tc.psum_pool(name="ppool", bufs=4) as ppool,
        ):
            PNt = cpool.tile([P, NBLK * 3], f32, tag="PN")
            nc.sync.dma_start(PNt[:, :], pred_rn[:, :])

            T = [cpool.tile([P, W], f32, tag=f"T{d}", name=f"T{d}") for d in range(3)]
            for d in range(3):
                nc.sync.dma_start(
                    T[d][:, :], target_t[d : d + 1, :].broadcast_to([P, W])
                )

            if OPSET in ("fused3", "fused4", "fused5"):
                sheets = [
                    cpool.tile([P, W], bf16, tag=f"sheet{s}", name=f"sheet{s}")
                    for s in range(NSHEET)
                ]
                for s in range(NSHEET):
                    nc.vector.memset(sheets[s][:, :], BIG)
            else:
                colmin = cpool.tile([P, W], bf16, tag="colmin")
                nc.vector.memset(colmin[:, :], BIG)
            rowmin = cpool.tile([P, NBLK], bf16, tag="rowmin")

            import contextlib

            if OPSET in ("fused", "fused2", "fused3", "fused4", "fused5"):
                OP1, OP2 = _register_ops()
            if OPSET == "fused5":
                Ibf = cpool.tile([P, P], bf16, tag="Ibf")
                nc.sync.dma_start(Ibf[:, :], ident_d[:, :])

            loop_ctx = tc.For_i(0, reps, 1) if reps > 1 else contextlib.nullcontext()
            with loop_ctx:
              if OPSET == "fused5":
                nb = NBLK if nblocks is None else nblocks
                for r in range(nb):
                    ws = slice(P * r, P * r + K)
                    bias = [PNt[:, 3 * r + d : 3 * r + d + 1] for d in range(3)]
                    dve_only = FMIX > 0 and (r % FMIX == FMIX - 1)
                    if dve_only:
                        A01 = wpool.tile([P, K], bf16, tag="A01")
                        nc.vector._custom_dve(
                            OP1, out=A01[:, :], in0=T[0][:, ws], in1=T[1][:, ws],
                            s0=bias[0], s1=bias[1],
                        )
                        nc.vector._custom_dve(
                            OP2, out=sheets[r % NSHEET][:, ws], in0=T[2][:, ws],
                            in1=A01[:, :], s0=bias[2], s1=BIG,
                            accum_out=rowmin[:, r : r + 1],
                        )
                        continue
                    A0 = apool.tile([P, K], bf16, tag="A0")
                    nc.scalar.activation(
                        A0[:, :], T[0][:, ws], Act.Abs, bias=bias[0], scale=1.0
                    )
                    A1 = apool.tile([P, K], bf16, tag="A1")
                    nc.scalar.activation(
                        A1[:, :], T[1][:, ws], Act.Abs, bias=bias[1], scale=1.0
                    )
                    S01p = ppool.tile([P, K], f32, tag="S01p")
                    nc.tensor.matmul(
                        S01p[:, :], Ibf[:, :], A0[:, :], start=True, stop=False
                    )
                    nc.tensor.matmul(
                        S01p[:, :], Ibf[:, :], A1[:, :], start=False, stop=True
                    )
                    nc.vector._custom_dve(
                        OP2, out=sheets[r % NSHEET][:, ws], in0=T[2][:, ws],
                        in1=S01p[:, :], s0=bias[2], s1=BIG,
                        accum_out=rowmin[:, r : r + 1],
                    )
              elif OPSET == "fused4":
                nb = NBLK if nblocks is None else nblocks
                for q in range(nb // 2):
                    r0 = 2 * q
                    blocks = (r0, r0 + 1)
                    dve_only = FMIX > 0 and (q % FMIX == FMIX - 1)
                    if dve_only:
                        for r in blocks:
                            ws = slice(P * r, P * r + K)
                            bias = [
                                PNt[:, 3 * r + d : 3 * r + d + 1] for d in range(3)
                            ]
                            A01 = wpool.tile([P, K], bf16, tag="A01")
                            nc.vector._custom_dve(
                                OP1, out=A01[:, :], in0=T[0][:, ws],
                                in1=T[1][:, ws], s0=bias[0], s1=bias[1],
                            )
                            nc.vector._custom_dve(
                                OP2, out=sheets[r % NSHEET][:, ws],
                                in0=T[2][:, ws], in1=A01[:, :],
                                s0=bias[2], s1=BIG,
                                accum_out=rowmin[:, r : r + 1],
                            )
                        continue
                    A0p = apool.tile([P, 2 * K], bf16, tag="A0p")
                    A1p = apool.tile([P, 2 * K], bf16, tag="A1p")
                    for i, r in enumerate(blocks):
                        ws = slice(P * r, P * r + K)
                        hs = slice(i * K, (i + 1) * K)
                        bias = [PNt[:, 3 * r + d : 3 * r + d + 1] for d in range(3)]
                        nc.scalar.activation(
                            A0p[:, hs], T[0][:, ws], Act.Abs, bias=bias[0], scale=1.0
                        )
                        nc.scalar.activation(
                            A1p[:, hs], T[1][:, ws], Act.Abs, bias=bias[1], scale=1.0
                        )
                    S01p = wpool.tile([P, 2 * K], bf16, tag="S01p")
                    nc.vector.tensor_tensor(
                        S01p[:, :], A0p[:, :], A1p[:, :], Alu.add
                    )
                    for i, r in enumerate(blocks):
                        ws = slice(P * r, P * r + K)
                        hs = slice(i * K, (i + 1) * K)
                        nz = PNt[:, 3 * r + 2 : 3 * r + 3]
                        nc.vector._custom_dve(
                            OP2, out=sheets[r % NSHEET][:, ws], in0=T[2][:, ws],
                            in1=S01p[:, hs], s0=nz, s1=BIG,
                            accum_out=rowmin[:, r : r + 1],
                        )
              else:
                for r in range(NBLK if nblocks is None else nblocks):
                    ws = slice(P * r, P * r + K)
                    biases = [PNt[:, 3 * r + d : 3 * r + d + 1] for d in range(3)]
                    if OPSET == "fused":
                        A01 = wpool.tile([P, K], bf16, tag="A01")
                        nc.vector._custom_dve(
                            OP1, out=A01[:, :], in0=T[0][:, ws], in1=T[1][:, ws],
                            s0=biases[0], s1=biases[1],
                        )
                        S = wpool.tile([P, K], bf16, tag="S")
                        nc.vector._custom_dve(
                            OP2, out=S[:, :], in0=T[2][:, ws], in1=A01[:, :],
                            s0=biases[2], s1=BIG,
                            accum_out=rowmin[:, r : r + 1],
                        )
                        nc.vector.tensor_tensor(
                            colmin[:, ws], colmin[:, ws], S[:, :], Alu.min
                        )
                        continue
                    if OPSET == "fused3":
                        A0 = apool.tile([P, K], bf16, tag="A0")
                        nc.scalar.activation(
                            A0[:, :], T[0][:, ws], Act.Abs, bias=biases[0], scale=1.0
                        )
                        A1 = apool.tile([P, K], bf16, tag="A1")
                        nc.scalar.activation(
                            A1[:, :], T[1][:, ws], Act.Abs, bias=biases[1], scale=1.0
                        )
                        S01 = wpool.tile([P, K], bf16, tag="S01")
                        nc.vector.tensor_tensor(
                            S01[:, :], A0[:, :], A1[:, :], Alu.add
                        )
                        nc.vector._custom_dve(
                            OP2, out=sheets[r % NSHEET][:, ws], in0=T[2][:, ws],
                            in1=S01[:, :], s0=biases[2], s1=BIG,
                            accum_out=rowmin[:, r : r + 1],
                        )
                        continue
                    if OPSET == "fused2":
                        A0 = apool.tile([P, K], bf16, tag="A0")
                        nc.scalar.activation(
                            A0[:, :], T[0][:, ws], Act.Abs, bias=biases[0], scale=1.0
                        )
                        A1 = apool.tile([P, K], bf16, tag="A1")
                        nc.scalar.activation(
                            A1[:, :], T[1][:, ws], Act.Abs, bias=biases[1], scale=1.0
                        )
                        S01 = wpool.tile([P, K], bf16, tag="S01")
                        nc.vector.tensor_tensor(
                            S01[:, :], A0[:, :], A1[:, :], Alu.add
                        )
                        S = wpool.tile([P, K], bf16, tag="S")
                        nc.vector._custom_dve(
                            OP2, out=S[:, :], in0=T[2][:, ws], in1=S01[:, :],
                            s0=biases[2], s1=BIG,
                            accum_out=rowmin[:, r : r + 1],
                        )
                        nc.vector.tensor_tensor(
                            colmin[:, ws], colmin[:, ws], S[:, :], Alu.min
                        )
                        continue
                    z_on_dve = ALT_Z > 0 and (r % ALT_Z == ALT_Z - 1)
                    y_on_dve = ALT_Y > 0 and (r % ALT_Y == ALT_Y - 1)

                    def make_abs(d, on_dve, tag):
                        Ad = apool.tile([P, K], bf16, tag=tag, name=tag)
                        if on_dve:
                            Dd = wpool.tile([P, K], bf16, tag=f"D{tag}", name=f"D{tag}")
                            nc.vector.tensor_scalar_add(
                                Dd[:, :], T[d][:, ws], biases[d]
                            )
                            nDd = wpool.tile(
                                [P, K], bf16, tag=f"nD{tag}", name=f"nD{tag}"
                            )
                            nc.vector.tensor_scalar_mul(nDd[:, :], Dd[:, :], -1.0)
                            nc.vector.tensor_tensor(
                                Ad[:, :], Dd[:, :], nDd[:, :], Alu.max
                            )
                        else:
                            nc.scalar.activation(
                                Ad[:, :], T[d][:, ws], Act.Abs, bias=biases[d], scale=1.0
                            )
                        return Ad

                    A0 = make_abs(0, False, "A0")
                    A1 = make_abs(1, y_on_dve, "A1")
                    A2 = make_abs(2, z_on_dve, "A2")
                    S01 = wpool.tile([P, K], bf16, tag="S01")
                    nc.vector.tensor_tensor(S01[:, :], A0[:, :], A1[:, :], Alu.add)
                    S = wpool.tile([P, K], bf16, tag="S")
                    nc.vector.tensor_tensor(S[:, :], S01[:, :], A2[:, :], Alu.add)
                    nc.vector.tensor_tensor(
                        colmin[:, ws], colmin[:, ws], S[:, :], Alu.min
                    )
                    if ROWMIN_MODE == "direct":
                        nc.vector.tensor_reduce(
                            rowmin[:, r : r + 1], S[:, :], mybir.AxisListType.X, Alu.min
                        )
                    elif ROWMIN_MODE == "scan":
                        Osc = wpool.tile([P, K // 2], bf16, tag="Osc")
                        nc.vector.tensor_tensor_scan(
                            Osc[:, :], S[:, : K // 2], S[:, K // 2 :], BIG,
                            Alu.min, Alu.min,
                        )
                        nc.vector.tensor_copy(
                            rowmin[:, r : r + 1], Osc[:, K // 2 - 1 : K // 2]
                        )
                    else:
                        F1 = wpool.tile([P, K // 2], bf16, tag="F1")
                        nc.vector.tensor_tensor(
                            F1[:, :], S[:, : K // 2], S[:, K // 2 :], Alu.min
                        )
                        if ROWMIN_MODE == "fold1":
                            nc.vector.tensor_reduce(
                                rowmin[:, r : r + 1], F1[:, :],
                                mybir.AxisListType.X, Alu.min,
                            )
                        else:
                            F2 = wpool.tile([P, K // 4], bf16, tag="F2")
                            nc.vector.tensor_tensor(
                                F2[:, :], F1[:, : K // 4], F1[:, K // 4 :], Alu.min
                            )
                            nc.vector.tensor_reduce(
                                rowmin[:, r : r + 1], F2[:, :],
                                mybir.AxisListType.X, Alu.min,
                            )

            nc.sync.dma_start(rowmin_d[:, :], rowmin[:, :])
            if OPSET in ("fused3", "fused4", "fused5"):
                for s in range(NSHEET):
                    nc.sync.dma_start(sheet_d[s][:, :], sheets[s][:, :])
            else:
                nc.sync.dma_start(colmin_d[:, :], colmin[:, :])

    nc.compile()
    return nc


def _sort_batch(pred_b, target_b):
    up = pred_b.sum(1)
    ut = target_b.sum(1)
    po = np.argsort(up, kind="stable")
    to = np.argsort(ut, kind="stable")
    return pred_b[po], target_b[to], up[po], ut[to]


def _shard(pred, target):
    HALF = K // 2
    in_maps = []
    meta = []
    for b in range(B):
        ps, ts, ups, uts = _sort_batch(pred[b], target[b])
        meta.append((ps, ts, ups, uts))
        for h in range(2):
            pr = ps[h * NPRED : (h + 1) * NPRED]  # [4096, 3]
            prn = np.ascontiguousarray(
                -pr.reshape(NBLK, P, 3).transpose(1, 0, 2).reshape(P, NBLK * 3)
            )
            G0 = NPRED * h + P // 2 - HALF
            Tpad = np.full((W, 3), SENTINEL, np.float32)
            lo, hi = max(0, G0), min(M, G0 + W)
            Tpad[lo - G0 : hi - G0] = ts[lo:hi]
            tt = np.ascontiguousarray(Tpad.T)  # [3, W]
            im = {"pred_rn": prn, "target_t": tt}
            if OPSET == "fused5":
                import ml_dtypes

                im["ident"] = np.eye(P, dtype=ml_dtypes.bfloat16)
            in_maps.append(im)
    return in_maps, meta


def _combine(results, meta):
    HALF = K // 2
    total = 0.0
    for b in range(B):
        ps, ts, ups, uts = meta[b]
        m_row = np.full(N, np.inf, np.float32)
        m_col = np.full(M, np.inf, np.float32)
        covA = np.full(M, N, np.int64)
        covB = np.full(M, -1, np.int64)
        for h in range(2):
            r = results[2 * b + h]
            rm = np.asarray(r["rowmin"]).astype(np.float32)  # [128, 32]
            # rowmin[p, blk] = pred at sorted rank NPRED*h + 128*blk + p
            gidx = NPRED * h + P * np.arange(NBLK)[None, :] + np.arange(P)[:, None]
            m_row[gidx.ravel()] = rm.ravel()
            if OPSET in ("fused3", "fused4", "fused5"):
                cm = np.min(
                    [
                        np.asarray(r[f"colmin{s}"]).astype(np.float32).min(axis=0)
                        for s in range(NSHEET)
                    ],
                    axis=0,
                )  # [W]
            else:
                cm = np.asarray(r["colmin"]).astype(np.float32).min(axis=0)  # [W]
            G0 = NPRED * h + P // 2 - HALF
            gt = G0 + np.arange(W)
            valid = (gt >= 0) & (gt < M)
            np.minimum.at(m_col, gt[valid], cm[valid])
            # coverage: block blk covers targets [G0+128*blk, G0+128*blk+K)
            for blk in range(NBLK):
                g = G0 + P * blk + np.arange(K)
                v = (g >= 0) & (g < M)
                covA[g[v]] = np.minimum(covA[g[v]], NPRED * h + P * blk)
                covB[g[v]] = np.maximum(covB[g[v]], NPRED * h + P * blk + P)
        # verification: rowmin
        h_arr = np.arange(N) // NPRED
        r_arr = (np.arange(N) % NPRED) // P
        wlo = NPRED * h_arr + P * r_arr + P // 2 - HALF
        whi = wlo + K
        gap_lo = np.where(wlo > 0, ups - uts[np.clip(wlo, 1, M) - 1], np.inf)
        gap_hi = np.where(whi < M, uts[np.clip(whi, 0, M - 1)] - ups, np.inf)
        ok_r = m_row <= np.minimum(gap_lo, gap_hi)
        for g in np.where(~ok_r)[0]:
            m_row[g] = np.abs(ps[g][None, :] - ts).sum(1).min()
        # verification: colmin
        gap_lo_c = np.where(covA > 0, uts - ups[np.clip(covA, 1, N) - 1], np.inf)
        gap_hi_c = np.where(covB < N, ups[np.clip(covB, 0, N - 1)] - uts, np.inf)
        ok_c = (m_col <= np.minimum(gap_lo_c, gap_hi_c)) & (covB > covA)
        for j in np.where(~ok_c)[0]:
            m_col[j] = np.abs(ts[j][None, :] - ps).sum(1).min()
        total += m_row.sum(dtype=np.float64) + m_col.sum(dtype=np.float64)
    return np.float32(total / B)


def kernel(pred, target):
    global _compiled
    from concourse import bass_utils

    pred = np.asarray(pred, dtype=np.float32)
    target = np.asarray(target, dtype=np.float32)
    if _compiled is None:
        _compiled = _build()
    in_maps, meta = _shard(pred, target)
    res = bass_utils.run_bass_kernel_spmd(
        _compiled, in_maps, core_ids=list(range(N_CORES))
    )
    return _combine(res.results, meta)
